# revision 5
# baseline (speedup 1.0000x reference)
"""Multi-head causal attention (B=2, S=2048, E=1024, H=16, D=64) on 8 TRN2 cores.

Sharding: core c handles batch b = c//4 and head-group g = c%4 (4 heads,
256 projection columns). Each core computes its partial out-projection
(ctx @ W_out[rows]); the host sums the 4 partials per batch.

Per-core pipeline (single SPMD program):
  1. X^T via PE identity-transpose                      [e, s] layout
  2. Q^T, K^T (fp32, q pre-scaled by 1/sqrt(D)), V (bf16, +ones column)
  3. per (head, q-tile of 512): S^T = K Q^T (fp32r), exp on ACT -> P^T
     (bf16), causal mask via gpsimd affine_select, ctx^T = [V|1].T @ P^T
     accumulated on PE; row 64 of ctx^T psum = softmax denominator;
     normalize via outer-product broadcast of 1/den.
  4. out = ctx @ W_out slice (fp32r), DMA out.
"""

import sys

if "/opt/trn_rl_repo" not in sys.path:
    sys.path.insert(0, "/opt/trn_rl_repo")

from contextlib import ExitStack

import numpy as np

import concourse.bacc as bacc
import concourse.mybir as mybir
import concourse.tile as tile
from concourse.bass_utils import run_bass_kernel_spmd
from concourse.masks import make_identity

P = 128
S = 2048
E = 1024
HC = 256          # head columns per core (4 heads x 64)
D = 64
NHC = 4           # heads per core
SC = S // P       # 16 s-chunks
ECH = E // P      # 8 e-chunks
QT = 512          # q-tile
NQT = S // QT     # 4
GK = 4            # k-blocks per exp group (one exp instr covers 4 psum banks)

F32 = mybir.dt.float32
F32R = mybir.dt.float32r
BF16 = mybir.dt.bfloat16
EXP = mybir.ActivationFunctionType.Exp


def build_nc():
    nc = bacc.Bacc("TRN2", target_bir_lowering=False)
    x = nc.dram_tensor("x", [S, E], F32, kind="ExternalInput")
    wq = nc.dram_tensor("wq", [E, HC], F32, kind="ExternalInput")
    wk = nc.dram_tensor("wk", [E, HC], F32, kind="ExternalInput")
    wv = nc.dram_tensor("wv", [E, HC], F32, kind="ExternalInput")
    wo = nc.dram_tensor("wo", [HC, E], F32, kind="ExternalInput")
    out = nc.dram_tensor("out", [S, E], F32, kind="ExternalOutput")

    with tile.TileContext(nc) as tc, ExitStack() as ctx:
        sb = ctx.enter_context(tc.tile_pool(name="sb", bufs=1))
        stage = ctx.enter_context(tc.tile_pool(name="stage", bufs=3))
        ps = ctx.enter_context(tc.tile_pool(name="ps", bufs=1, space="PSUM"))

        ident = sb.tile([P, P], F32)
        make_identity(nc, ident[:])
        ones = sb.tile([1, D], F32)
        nc.gpsimd.memset(ones[:], 1.0)

        wq_sb = sb.tile([P, ECH, HC], F32R)
        wk_sb = sb.tile([P, ECH, HC], F32R)
        wv_sb = sb.tile([P, ECH, HC], F32R)
        wo_sb = sb.tile([P, 2, E], F32R)
        nc.sync.dma_start(wq_sb[:], wq.rearrange("(eo p) n -> p eo n", p=P).bitcast(F32R))
        nc.sync.dma_start(wk_sb[:], wk.rearrange("(eo p) n -> p eo n", p=P).bitcast(F32R))
        nc.sync.dma_start(wv_sb[:], wv.rearrange("(eo p) n -> p eo n", p=P).bitcast(F32R))
        nc.sync.dma_start(wo_sb[:], wo.rearrange("(c p) m -> p c m", p=P).bitcast(F32R))

        # ---- phase 1: X^T [e, s] via PE transpose ----
        xT = sb.tile([P, ECH, S], F32R)
        for sc in range(SC):
            xs = stage.tile([P, E], F32, tag="xs")
            nc.sync.dma_start(xs[:], x[sc * P : (sc + 1) * P, :])
            for ec in range(ECH):
                tp = ps.tile([P, P], F32, tag="mm", bufs=2)
                nc.tensor.transpose(tp[:], xs[:, ec * P : (ec + 1) * P], ident[:])
                nc.vector.tensor_copy(xT[:, ec, sc * P : (sc + 1) * P], tp[:])

        # ---- phase 2: projections ----
        # qT/kT: [n, s] packed 2 heads per 128 partitions; q scaled by 1/8.
        qT = sb.tile([P, 2, S], F32R)
        kT = sb.tile([P, 2, S], F32R)
        for hp in range(2):
            for st in range(NQT):
                pq = ps.tile([P, QT], F32, tag="mm", bufs=2)
                for ec in range(ECH):
                    nc.tensor.matmul(
                        pq[:],
                        wq_sb[:, ec, hp * P : (hp + 1) * P],
                        xT[:, ec, st * QT : (st + 1) * QT],
                        start=(ec == 0),
                        stop=(ec == ECH - 1),
                    )
                nc.scalar.mul(qT[:, hp, st * QT : (st + 1) * QT], pq[:], 0.125)
                pk = ps.tile([P, QT], F32, tag="mm", bufs=2)
                for ec in range(ECH):
                    nc.tensor.matmul(
                        pk[:],
                        wk_sb[:, ec, hp * P : (hp + 1) * P],
                        xT[:, ec, st * QT : (st + 1) * QT],
                        start=(ec == 0),
                        stop=(ec == ECH - 1),
                    )
                nc.vector.tensor_copy(kT[:, hp, st * QT : (st + 1) * QT], pk[:])

        # V in [s, (head, d+1)] layout, bf16, ones in column D for the
        # softmax denominator row of the ctx^T matmul.
        vC = sb.tile([P, SC, NHC, D + 1], BF16)
        nc.gpsimd.memset(vC[:, :, :, D], 1.0)
        for sc in range(SC):
            pv = ps.tile([P, HC], F32, tag="mm", bufs=2)
            for ec in range(ECH):
                nc.tensor.matmul(
                    pv[:],
                    xT[:, ec, sc * P : (sc + 1) * P],
                    wv_sb[:, ec, :],
                    start=(ec == 0),
                    stop=(ec == ECH - 1),
                )
            for h in range(NHC):
                nc.vector.tensor_copy(
                    vC[:, sc, h, 0:D], pv[:, h * D : (h + 1) * D]
                )

        # ---- phase 3: attention ----
        ctxT = sb.tile([P, 2, S], F32R)
        for h in range(NHC):
            hp, sub = h // 2, (h % 2) * D
            for qt in range(NQT):
                ngroups = qt + 1
                nkb = GK * ngroups
                cps = ps.tile([D + 1, QT], F32, tag="ctx", bufs=2)
                for g in range(ngroups):
                    stp = ps.tile([P, GK, QT], F32, tag="st", bufs=1)
                    for j in range(GK):
                        kb = g * GK + j
                        nc.tensor.matmul(
                            stp[:, j, :],
                            kT[sub : sub + D, hp, kb * P : (kb + 1) * P],
                            qT[sub : sub + D, hp, qt * QT : (qt + 1) * QT],
                            start=True,
                            stop=True,
                        )
                    pt = stage.tile([P, GK, QT], BF16, tag="pt", bufs=4)
                    nc.scalar.activation(pt[:], stp[:], EXP)
                    if g == qt:
                        # diagonal group: zero where q < k
                        nc.gpsimd.affine_select(
                            out=pt[:],
                            in_=pt[:],
                            compare_op=mybir.AluOpType.is_ge,
                            fill=0.0,
                            base=0,
                            channel_multiplier=-1,
                            pattern=[[-P, GK], [1, QT]],
                        )
                    for j in range(GK):
                        kb = g * GK + j
                        nc.tensor.matmul(
                            cps[:],
                            vC[:, kb, h, :],
                            pt[:, j, :],
                            start=(kb == 0),
                            stop=(kb == nkb - 1),
                        )
                recip = stage.tile([1, QT], F32, tag="recip", bufs=2)
                nc.vector.reciprocal(recip[:], cps[D : D + 1, :])
                bc = ps.tile([D, QT], F32, tag="mm", bufs=2)
                nc.tensor.matmul(bc[:], ones[:], recip[:], start=True, stop=True)
                bc_sb = stage.tile([D, QT], F32, tag="bc_sb", bufs=2)
                nc.vector.tensor_copy(bc_sb[:], bc[:])
                nc.vector.tensor_mul(
                    ctxT[sub : sub + D, hp, qt * QT : (qt + 1) * QT],
                    cps[0:D, :],
                    bc_sb[:],
                )

        # ---- phase 4: out-projection (partial; host sums over cores) ----
        for sc in range(SC):
            for mh in range(2):
                po = ps.tile([P, QT], F32, tag="mm", bufs=2)
                for c2 in range(2):
                    nc.tensor.matmul(
                        po[:],
                        ctxT[:, c2, sc * P : (sc + 1) * P],
                        wo_sb[:, c2, mh * QT : (mh + 1) * QT],
                        start=(c2 == 0),
                        stop=(c2 == 1),
                    )
                ob = stage.tile([P, QT], F32, tag="ob", bufs=3)
                nc.vector.tensor_copy(ob[:], po[:])
                nc.sync.dma_start(
                    out[sc * P : (sc + 1) * P, mh * QT : (mh + 1) * QT], ob[:]
                )

    nc.finalize()
    return nc


_NC = None


def _get_nc():
    global _NC
    if _NC is None:
        _NC = build_nc()
    return _NC


def make_in_maps(X_emb, W_q, W_k, W_v, W_out):
    in_maps = []
    for c in range(8):
        b, g = c // 4, c % 4
        cols = slice(g * HC, (g + 1) * HC)
        in_maps.append(
            {
                "x": np.ascontiguousarray(X_emb[b], dtype=np.float32),
                "wq": np.ascontiguousarray(W_q[:, cols], dtype=np.float32),
                "wk": np.ascontiguousarray(W_k[:, cols], dtype=np.float32),
                "wv": np.ascontiguousarray(W_v[:, cols], dtype=np.float32),
                "wo": np.ascontiguousarray(W_out[cols, :], dtype=np.float32),
            }
        )
    return in_maps


def combine_outputs(outs):
    return np.stack(
        [
            outs[0] + outs[1] + outs[2] + outs[3],
            outs[4] + outs[5] + outs[6] + outs[7],
        ]
    ).astype(np.float32)


def kernel(X_emb, W_q, W_k, W_v, W_out, _spmd_kwargs=None):
    X_emb = np.asarray(X_emb, dtype=np.float32)
    nc = _get_nc()
    in_maps = make_in_maps(X_emb, W_q, W_k, W_v, W_out)
    res = run_bass_kernel_spmd(nc, in_maps, core_ids=list(range(8)), **(_spmd_kwargs or {}))
    outs = [res.results[c]["out"] for c in range(8)]
    full = combine_outputs(outs)
    if _spmd_kwargs:
        kernel.last_result = res
    return full
